# revision 1
# baseline (speedup 1.0000x reference)
"""v4: bf16 stationary operands (fast weight load), f32r moving; ACT=exp only;
batched reciprocal; DVE evacs."""
import numpy as np
import concourse.bass as bass
import concourse.mybir as mybir
import concourse.tile as tile
from concourse.masks import make_identity

dt = mybir.dt
F32 = dt.float32
F32R = dt.float32r
BF16 = dt.bfloat16
AF = mybir.ActivationFunctionType

B = 4
T = 577
D = 768
H = 12
HD = 64
EQK = 1536
SCALE = HD ** -0.5
NTOK = B * T

TT = [(i * 128, min(128, T - i * 128)) for i in range((T + 127) // 128)]
TP = 578
ICH = [(0, 320), (320, 258)]
ICHV = [(0, 320), (320, 257)]
ECH = [(0, 384), (384, 384)]
DT = 6


def build(nbatch=B, sd="bf16", md="f32r", attn16=False):
    SD = {"bf16": BF16, "f32r": F32R, "f32": F32}[sd]   # stationary dtype
    MD = {"bf16": BF16, "f32r": F32R, "f32": F32}[md]   # moving dtype
    nc = bass.Bass()
    x_d = nc.dram_tensor("x", [NTOK, D], F32, kind="ExternalInput")
    qkwT_d = nc.dram_tensor("qkwT", [D, EQK], F32, kind="ExternalInput")
    vwT_d = nc.dram_tensor("vwT", [D, D], F32, kind="ExternalInput")
    pwT_d = nc.dram_tensor("pwT", [D + 1, D], F32, kind="ExternalInput")
    qkb_d = nc.dram_tensor("qkb", [128, 12], F32, kind="ExternalInput")
    y_d = nc.dram_tensor("y", [NTOK, D], F32, kind="ExternalOutput")

    ESD = BF16 if attn16 else MD   # dtype of exp(S) and v (MM3 operands)
    deep = (SD == BF16 and MD == BF16)
    from contextlib import ExitStack
    with tile.TileContext(nc) as tc, ExitStack() as ctx:
        wpool = ctx.enter_context(tc.tile_pool(name="wpool", bufs=1))
        stg = ctx.enter_context(tc.tile_pool(name="stg", bufs=2))

        ident = wpool.tile([128, 128], F32, tag="ident")
        make_identity(nc, ident[:])

        ones_row_f = wpool.tile([1, T], F32, tag="ones_row_f")
        nc.gpsimd.memset(ones_row_f[:], 1.0)
        ones_row = wpool.tile([1, T], SD, tag="ones_row")
        nc.vector.tensor_copy(ones_row[:], ones_row_f[:])
        zcol_f = wpool.tile([128, 1], F32, tag="zcol_f")
        nc.gpsimd.memset(zcol_f[:], 0.0)
        ones_col_f = wpool.tile([128, 1], F32, tag="ones_col_f")
        nc.gpsimd.memset(ones_col_f[:], 1.0)
        ones_col = wpool.tile([128, 1], ESD, tag="ones_col")
        nc.vector.tensor_copy(ones_col[:], ones_col_f[:])

        qkb_sb = wpool.tile([128, 12], F32, tag="qkb")
        nc.sync.dma_start(qkb_sb[:], qkb_d[:])

        qkwT = []   # stationary (lhsT of MM1a)
        for dti in range(DT):
            w = wpool.tile([128, EQK], SD, tag=f"qkwT{dti}", name=f"qkwT{dti}")
            for half in range(2):
                s = stg.tile([128, D], F32, tag="wstage", name=f"st{dti}_{half}")
                nc.sync.dma_start(s[:], qkwT_d[dti * 128:(dti + 1) * 128, half * D:(half + 1) * D])
                nc.vector.tensor_copy(w[:, half * D:(half + 1) * D], s[:])
            qkwT.append(w)
        vwT = []    # moving (rhs of MM1b)
        for dti in range(DT):
            w = wpool.tile([128, D], MD, tag=f"vwT{dti}", name=f"vwT{dti}")
            s = stg.tile([128, D], F32, tag="wstage", name=f"sv{dti}")
            nc.sync.dma_start(s[:], vwT_d[dti * 128:(dti + 1) * 128, :])
            nc.vector.tensor_copy(w[:], s[:])
            vwT.append(w)
        pwT = []    # moving (rhs of MM4)
        for dti in range(DT):
            w = wpool.tile([128, D], MD, tag=f"pwT{dti}", name=f"pwT{dti}")
            s = stg.tile([128, D], F32, tag="wstage", name=f"sp{dti}")
            nc.sync.dma_start(s[:], pwT_d[dti * 128:(dti + 1) * 128, :])
            nc.vector.tensor_copy(w[:], s[:])
            pwT.append(w)
        pb_sb = wpool.tile([1, D], MD, tag="pb")
        s = stg.tile([128, D], F32, tag="wstage", name="spb")
        nc.sync.dma_start(s[0:1, :], pwT_d[D:D + 1, :])
        nc.vector.tensor_copy(pb_sb[:], s[0:1, :])

        xin = ctx.enter_context(tc.tile_pool(name="xin", bufs=3 if deep else 2))
        xT_p = ctx.enter_context(tc.tile_pool(name="xT", bufs=1))
        qkT_p = ctx.enter_context(tc.tile_pool(name="qkT", bufs=1))
        v_p = ctx.enter_context(tc.tile_pool(name="v", bufs=1))
        es_p = ctx.enter_context(tc.tile_pool(name="es", bufs=1))
        oT_p = ctx.enter_context(tc.tile_pool(name="oT", bufs=2 if deep else 1))
        nrm_p = ctx.enter_context(tc.tile_pool(name="nrm", bufs=4 if deep else 3))
        den_p = ctx.enter_context(tc.tile_pool(name="den", bufs=2 if deep else 1))
        yout = ctx.enter_context(tc.tile_pool(name="yout", bufs=3 if deep else 2))
        drp = ctx.enter_context(tc.tile_pool(name="dr", bufs=2, space="DRAM"))

        ps_s = ctx.enter_context(tc.tile_pool(name="ps_s", bufs=3, space="PSUM"))
        ps_mm = ctx.enter_context(tc.tile_pool(name="ps_mm", bufs=2, space="PSUM"))
        ps_t = ctx.enter_context(tc.tile_pool(name="ps_t", bufs=1, space="PSUM"))
        ps_o = ctx.enter_context(tc.tile_pool(name="ps_o", bufs=2, space="PSUM"))

        state = {}

        def stage1(b):
            x0 = b * T
            # xT in BOTH dtypes: MD copy (moving for MM1a) and SD copy (stationary for MM1b)
            xT = [xT_p.tile([128, TP], MD, tag=f"xT{dti}", name=f"xT{dti}_{b}") for dti in range(DT)]
            if SD == MD:
                xS = xT
            else:
                xS = [xT_p.tile([128, TP], SD, tag=f"xS{dti}", name=f"xS{dti}_{b}") for dti in range(DT)]
            for ti, (ts_, P) in enumerate(TT):
                xt = xin.tile([128, D], F32, tag="x_in", name=f"x_{b}_{ti}")
                nc.sync.dma_start(xt[0:P, :], x_d[x0 + ts_: x0 + ts_ + P, :])
                for dti in range(DT):
                    pt = ps_t.tile([128, 128], F32, tag="ps_t", name=f"pt_{b}_{ti}_{dti}")
                    nc.tensor.transpose(pt[:, 0:P], xt[0:P, dti * 128:(dti + 1) * 128], ident[0:P, 0:P])
                    nc.vector.tensor_copy(xT[dti][:, ts_:ts_ + P], pt[:, 0:P])
                    if SD != MD:
                        nc.vector.tensor_copy(xS[dti][:, ts_:ts_ + P], pt[:, 0:P])
            for dti in range(DT):
                nc.vector.tensor_copy(xT[dti][:, T:TP], zcol_f[:])

            # MM1a: qkT; q e-tiles (0-5) in MD (moving for MM2 rhs), k e-tiles (6-11) in SD (stationary)
            qkT = [qkT_p.tile([128, TP], MD if et < 6 else SD, tag=f"qkT{et}", name=f"qkT{et}_{b}")
                   for et in range(12)]
            for et in range(12):
                for (cs, cw) in ICH:
                    pm = ps_mm.tile([128, 512], F32, tag="ps_mm", name=f"pma_{b}_{et}_{cs}")
                    for dti in range(DT):
                        nc.tensor.matmul(pm[:, 0:cw],
                                         qkwT[dti][:, et * 128:(et + 1) * 128],
                                         xT[dti][:, cs:cs + cw],
                                         start=(dti == 0), stop=(dti == DT - 1))
                    nc.vector.tensor_scalar_add(qkT[et][:, cs:cs + cw], pm[:, 0:cw],
                                                qkb_sb[:, et:et + 1])

            # MM1b: v token-major in SD; per-head contiguous copies + ones col
            v_sb = [v_p.tile([128, H * (HD + 1)], ESD, tag=f"v{ti}", name=f"v{ti}_{b}") for ti in range(len(TT))]
            for ti, (ts_, P) in enumerate(TT):
                vv = v_sb[ti].rearrange("p (h c) -> p h c", c=HD + 1)
                nc.vector.tensor_copy(vv[0:P, :, HD:HD + 1], ones_col[0:P, :].to_broadcast((P, H, 1)))
                for ci, (cs, cw) in enumerate(ECH):
                    pm = ps_mm.tile([128, 512], F32, tag="ps_mm", name=f"pmb_{b}_{ti}_{ci}")
                    for dti in range(DT):
                        nc.tensor.matmul(pm[0:P, 0:cw],
                                         xS[dti][:, ts_:ts_ + P],
                                         vwT[dti][:, cs:cs + cw],
                                         start=(dti == 0), stop=(dti == DT - 1))
                    for hh in range(6):
                        h = ci * 6 + hh
                        nc.vector.tensor_copy(v_sb[ti][0:P, h * (HD + 1):h * (HD + 1) + HD],
                                              pm[0:P, hh * HD:(hh + 1) * HD])

            state[b] = (xT, xS, qkT, v_sb)

        def attn(b):
            x0 = b * T
            xT, xS, qkT, v_sb = state.pop(b)
            # attention
            oT = [oT_p.tile([128, TP], SD, tag=f"oT{dti}", name=f"oT{dti}_{b}") for dti in range(DT)]
            rdr_den = drp.tile([12, TP], F32, tag="rdr_den", name=f"rdrden_{b}")
            for h in range(H):
                g, par = h // 2, (h % 2) * 64
                qt = qkT[g]
                kt = qkT[6 + g]
                es = [es_p.tile([128, TP], ESD, tag=f"es{ji}_{h % (3 if (deep or attn16) else 2)}", name=f"es{ji}_{b}_{h}") for ji in range(len(TT))]
                for ji, (js, JP) in enumerate(TT):
                    for (cs, cw) in ICH:
                        pss = ps_s.tile([128, 320], F32, tag="ps_s", name=f"pss_{b}_{h}_{ji}_{cs}")
                        nc.tensor.matmul(pss[0:JP, 0:cw],
                                         kt[par:par + 64, js:js + JP],
                                         qt[par:par + 64, cs:cs + cw],
                                         start=True, stop=True)
                        nc.scalar.activation(es[ji][0:JP, cs:cs + cw], pss[0:JP, 0:cw],
                                             AF.Exp, scale=SCALE)
                for (cs, cw), (_, cwv) in zip(ICH, ICHV):
                    po = ps_o.tile([128, 320], F32, tag="ps_o", name=f"po_{b}_{h}_{cs}")
                    for ji, (js, JP) in enumerate(TT):
                        nc.tensor.matmul(po[0:HD + 1, 0:cw],
                                         v_sb[ji][0:JP, h * (HD + 1):(h + 1) * (HD + 1)],
                                         es[ji][0:JP, cs:cs + cw],
                                         start=(ji == 0), stop=(ji == len(TT) - 1))
                    # evac unnormalized o and the denominator row (via partition-0 tile -> DRAM)
                    nc.vector.tensor_copy(oT[g][par:par + 64, cs:cs + cwv], po[0:HD, 0:cwv])
                    dh = nrm_p.tile([1, 320], F32, tag="dh", name=f"dh_{b}_{h}_{cs}")
                    nc.vector.tensor_copy(dh[:, 0:cwv], po[HD:HD + 1, 0:cwv])
                    nc.sync.dma_start(rdr_den[h:h + 1, cs:cs + cwv], dh[:, 0:cwv])

            # batched reciprocal + per-head broadcast + in-place normalize
            den = den_p.tile([12, TP], F32, tag="den", name=f"den_{b}")
            nc.sync.dma_start(den[:, 0:T], rdr_den[:, 0:T])
            rec = den_p.tile([12, TP], F32, tag="rec", name=f"rec_{b}")
            nc.vector.reciprocal(rec[:, 0:T], den[:, 0:T])
            rdr = drp.tile([12, TP], F32, tag="rdr", name=f"rdr_{b}")
            nc.sync.dma_start(rdr[:, 0:T], rec[:, 0:T])
            for h in range(H):
                g, par = h // 2, (h % 2) * 64
                bc = nrm_p.tile([128, TP], F32, tag="bc", name=f"bc_{b}_{h}")
                nc.sync.dma_start(bc[par:par + 64, 0:T], rdr[h:h + 1, 0:T].to_broadcast((64, T)))
                nc.vector.tensor_tensor(oT[g][par:par + 64, 0:T],
                                        oT[g][par:par + 64, 0:T],
                                        bc[par:par + 64, 0:T], mybir.AluOpType.mult)

            # MM4
            for ti, (ts_, P) in enumerate(TT):
                ys = yout.tile([128, D], F32, tag="y_sb", name=f"ys_{b}_{ti}")
                for (cs, cw) in ECH:
                    pm = ps_mm.tile([128, 512], F32, tag="ps_mm", name=f"pmc_{b}_{ti}_{cs}")
                    for dti in range(DT):
                        nc.tensor.matmul(pm[0:P, 0:cw],
                                         oT[dti][:, ts_:ts_ + P],
                                         pwT[dti][:, cs:cs + cw],
                                         start=(dti == 0), stop=False)
                    nc.tensor.matmul(pm[0:P, 0:cw],
                                     ones_row[:, ts_:ts_ + P],
                                     pb_sb[:, cs:cs + cw],
                                     start=False, stop=True)
                    nc.vector.tensor_copy(ys[0:P, cs:cs + cw], pm[0:P, 0:cw])
                nc.sync.dma_start(y_d[x0 + ts_: x0 + ts_ + P, :], ys[0:P, :])


        stage1(0)
        for b in range(1, nbatch):
            stage1(b)
            attn(b - 1)
        attn(nbatch - 1)
    return nc


def host_inputs(x_c, qkv_w, qkv_b, proj_w, proj_b):
    qkwT = np.ascontiguousarray(qkv_w[0:EQK].T)
    vwT = np.ascontiguousarray(qkv_w[EQK:2304].T)
    b_v = qkv_b[EQK:2304]
    pb_aug = proj_b + b_v @ proj_w.T
    pwT = np.concatenate([proj_w.T, pb_aug[None, :]], axis=0)
    qkb = np.ascontiguousarray(qkv_b[0:EQK].reshape(12, 128).T)
    return {
        "x": np.ascontiguousarray(x_c.reshape(NTOK, D)).astype(np.float32),
        "qkwT": qkwT.astype(np.float32),
        "vwT": vwT.astype(np.float32),
        "pwT": pwT.astype(np.float32),
        "qkb": qkb.astype(np.float32),
    }


import sys as _sys
import numpy as _np

def _split_waits(nc, max_waits=1):
    import concourse.mybir as mybir
    nid = [0]
    for f in nc.m.functions:
        for bb in f.blocks:
            newlist = []; changed = False
            for ins in bb.instructions:
                si = getattr(ins, 'sync_info', None)
                if si is not None and si.on_wait is not None and len(si.on_wait) > max_waits:
                    waits = list(si.on_wait)
                    extra, keep = waits[:-max_waits], waits[-max_waits:]
                    for i in range(0, len(extra), max_waits):
                        nop = mybir.InstNoOp(name=f"I-ws-{nid[0]}", ins=[], outs=[],
                            engine=ins.engine,
                            sync_info=mybir.SyncInfo(on_wait=extra[i:i+max_waits], on_update=[]))
                        nid[0] += 1; newlist.append(nop); changed = True
                    si.on_wait = keep; ins.sync_info = si
                newlist.append(ins)
            if changed:
                bb.instructions = newlist


_NC_CACHE = {}

def _get_nc():
    if "nc" not in _NC_CACHE:
        nc = build(nbatch=B, sd="f32r", md="f32r")
        _split_waits(nc)
        _NC_CACHE["nc"] = nc
    return _NC_CACHE["nc"]


def kernel(x, qkv_w, qkv_b, proj_w, proj_b):
    """Full inputs in ([32,577,768] etc.), full output out.

    Data-parallel over batch: 32 batches -> 8 NeuronCores x 4 each. Weights
    replicated (host-side transpose is layout prep only); all compute on
    device (Bass/Tile, fp32r matmuls with fp32 accumulation, software-
    pipelined batch stages).
    """
    x = _np.asarray(x, dtype=_np.float32)
    qkv_w = _np.asarray(qkv_w, dtype=_np.float32)
    qkv_b = _np.asarray(qkv_b, dtype=_np.float32)
    proj_w = _np.asarray(proj_w, dtype=_np.float32)
    proj_b = _np.asarray(proj_b, dtype=_np.float32)
    from concourse.bass_utils import run_bass_kernel_spmd
    nc = _get_nc()
    in_maps = [host_inputs(x[c * B:(c + 1) * B], qkv_w, qkv_b, proj_w, proj_b)
               for c in range(8)]
    res = run_bass_kernel_spmd(nc, in_maps, list(range(8)))
    y = _np.concatenate([res.results[c]["y"].reshape(B, T, D) for c in range(8)], axis=0)
    return y.astype(_np.float32)



# revision 2
# speedup vs baseline: 1.5907x; 1.5907x over previous
"""v4: bf16 stationary operands (fast weight load), f32r moving; ACT=exp only;
batched reciprocal; DVE evacs."""
import numpy as np
import concourse.bass as bass
import concourse.mybir as mybir
import concourse.tile as tile
from concourse.masks import make_identity

dt = mybir.dt
F32 = dt.float32
F32R = dt.float32r
BF16 = dt.bfloat16
AF = mybir.ActivationFunctionType

B = 4
T = 577
D = 768
H = 12
HD = 64
EQK = 1536
SCALE = HD ** -0.5
NTOK = B * T

TT = [(i * 128, min(128, T - i * 128)) for i in range((T + 127) // 128)]
TP = 578
ICH = [(0, 320), (320, 258)]
ICHV = [(0, 320), (320, 257)]
ECH = [(0, 384), (384, 384)]
DT = 6


def build(nbatch=B, sd="bf16", md="f32r", attn16=False):
    SD = {"bf16": BF16, "f32r": F32R, "f32": F32}[sd]   # stationary dtype
    MD = {"bf16": BF16, "f32r": F32R, "f32": F32}[md]   # moving dtype
    nc = bass.Bass()
    x_d = nc.dram_tensor("x", [NTOK, D], F32, kind="ExternalInput")
    qkwT_d = nc.dram_tensor("qkwT", [D, EQK], F32, kind="ExternalInput")
    vwT_d = nc.dram_tensor("vwT", [D, D], F32, kind="ExternalInput")
    pwT_d = nc.dram_tensor("pwT", [D + 1, D], F32, kind="ExternalInput")
    qkb_d = nc.dram_tensor("qkb", [128, 12], F32, kind="ExternalInput")
    y_d = nc.dram_tensor("y", [NTOK, D], F32, kind="ExternalOutput")

    ESD = BF16 if attn16 else MD   # dtype of exp(S) and v (MM3 operands)
    deep = (SD == BF16 and MD == BF16)
    from contextlib import ExitStack
    with tile.TileContext(nc) as tc, ExitStack() as ctx:
        wpool = ctx.enter_context(tc.tile_pool(name="wpool", bufs=1))
        stg = ctx.enter_context(tc.tile_pool(name="stg", bufs=2))

        ident = wpool.tile([128, 128], F32, tag="ident")
        make_identity(nc, ident[:])

        ones_row_f = wpool.tile([1, T], F32, tag="ones_row_f")
        nc.gpsimd.memset(ones_row_f[:], 1.0)
        ones_row = wpool.tile([1, T], SD, tag="ones_row")
        nc.vector.tensor_copy(ones_row[:], ones_row_f[:])
        zcol_f = wpool.tile([128, 1], F32, tag="zcol_f")
        nc.gpsimd.memset(zcol_f[:], 0.0)
        ones_col_f = wpool.tile([128, 1], F32, tag="ones_col_f")
        nc.gpsimd.memset(ones_col_f[:], 1.0)
        ones_col = wpool.tile([128, 1], ESD, tag="ones_col")
        nc.vector.tensor_copy(ones_col[:], ones_col_f[:])

        qkb_sb = wpool.tile([128, 12], F32, tag="qkb")
        nc.sync.dma_start(qkb_sb[:], qkb_d[:])

        qkwT = []   # stationary (lhsT of MM1a)
        for dti in range(DT):
            w = wpool.tile([128, EQK], SD, tag=f"qkwT{dti}", name=f"qkwT{dti}")
            for half in range(2):
                s = stg.tile([128, D], F32, tag="wstage", name=f"st{dti}_{half}")
                nc.sync.dma_start(s[:], qkwT_d[dti * 128:(dti + 1) * 128, half * D:(half + 1) * D])
                nc.vector.tensor_copy(w[:, half * D:(half + 1) * D], s[:])
            qkwT.append(w)
        vwT = []    # moving (rhs of MM1b)
        for dti in range(DT):
            w = wpool.tile([128, D], MD, tag=f"vwT{dti}", name=f"vwT{dti}")
            s = stg.tile([128, D], F32, tag="wstage", name=f"sv{dti}")
            nc.sync.dma_start(s[:], vwT_d[dti * 128:(dti + 1) * 128, :])
            nc.vector.tensor_copy(w[:], s[:])
            vwT.append(w)
        pwT = []    # moving (rhs of MM4)
        for dti in range(DT):
            w = wpool.tile([128, D], MD, tag=f"pwT{dti}", name=f"pwT{dti}")
            s = stg.tile([128, D], F32, tag="wstage", name=f"sp{dti}")
            nc.sync.dma_start(s[:], pwT_d[dti * 128:(dti + 1) * 128, :])
            nc.vector.tensor_copy(w[:], s[:])
            pwT.append(w)
        pb_sb = wpool.tile([1, D], MD, tag="pb")
        s = stg.tile([128, D], F32, tag="wstage", name="spb")
        nc.sync.dma_start(s[0:1, :], pwT_d[D:D + 1, :])
        nc.vector.tensor_copy(pb_sb[:], s[0:1, :])

        xin = ctx.enter_context(tc.tile_pool(name="xin", bufs=3 if deep else 2))
        xT_p = ctx.enter_context(tc.tile_pool(name="xT", bufs=1))
        qkT_p = ctx.enter_context(tc.tile_pool(name="qkT", bufs=1))
        v_p = ctx.enter_context(tc.tile_pool(name="v", bufs=1))
        es_p = ctx.enter_context(tc.tile_pool(name="es", bufs=1))
        oT_p = ctx.enter_context(tc.tile_pool(name="oT", bufs=2 if deep else 1))
        nrm_p = ctx.enter_context(tc.tile_pool(name="nrm", bufs=4 if deep else 3))
        den_p = ctx.enter_context(tc.tile_pool(name="den", bufs=2 if deep else 1))
        yout = ctx.enter_context(tc.tile_pool(name="yout", bufs=3 if deep else 2))
        drp = ctx.enter_context(tc.tile_pool(name="dr", bufs=2, space="DRAM"))

        ps_s = ctx.enter_context(tc.tile_pool(name="ps_s", bufs=3, space="PSUM"))
        ps_mm = ctx.enter_context(tc.tile_pool(name="ps_mm", bufs=2, space="PSUM"))
        ps_t = ctx.enter_context(tc.tile_pool(name="ps_t", bufs=1, space="PSUM"))
        ps_o = ctx.enter_context(tc.tile_pool(name="ps_o", bufs=2, space="PSUM"))

        state = {}

        def stage1(b):
            x0 = b * T
            # xT in BOTH dtypes: MD copy (moving for MM1a) and SD copy (stationary for MM1b)
            xT = [xT_p.tile([128, TP], MD, tag=f"xT{dti}", name=f"xT{dti}_{b}") for dti in range(DT)]
            if SD == MD:
                xS = xT
            else:
                xS = [xT_p.tile([128, TP], SD, tag=f"xS{dti}", name=f"xS{dti}_{b}") for dti in range(DT)]
            for ti, (ts_, P) in enumerate(TT):
                xt = xin.tile([128, D], F32, tag="x_in", name=f"x_{b}_{ti}")
                nc.sync.dma_start(xt[0:P, :], x_d[x0 + ts_: x0 + ts_ + P, :])
                for dti in range(DT):
                    pt = ps_t.tile([128, 128], F32, tag="ps_t", name=f"pt_{b}_{ti}_{dti}")
                    nc.tensor.transpose(pt[:, 0:P], xt[0:P, dti * 128:(dti + 1) * 128], ident[0:P, 0:P])
                    nc.vector.tensor_copy(xT[dti][:, ts_:ts_ + P], pt[:, 0:P])
                    if SD != MD:
                        nc.vector.tensor_copy(xS[dti][:, ts_:ts_ + P], pt[:, 0:P])
            for dti in range(DT):
                nc.vector.tensor_copy(xT[dti][:, T:TP], zcol_f[:])

            # MM1a: qkT; q e-tiles (0-5) in MD (moving for MM2 rhs), k e-tiles (6-11) in SD (stationary)
            qkT = [qkT_p.tile([128, TP], MD if et < 6 else SD, tag=f"qkT{et}", name=f"qkT{et}_{b}")
                   for et in range(12)]
            for et in range(12):
                for (cs, cw) in ICH:
                    pm = ps_mm.tile([128, 512], F32, tag="ps_mm", name=f"pma_{b}_{et}_{cs}")
                    for dti in range(DT):
                        nc.tensor.matmul(pm[:, 0:cw],
                                         qkwT[dti][:, et * 128:(et + 1) * 128],
                                         xT[dti][:, cs:cs + cw],
                                         start=(dti == 0), stop=(dti == DT - 1))
                    nc.vector.tensor_scalar_add(qkT[et][:, cs:cs + cw], pm[:, 0:cw],
                                                qkb_sb[:, et:et + 1])

            # MM1b: v token-major in SD; per-head contiguous copies + ones col
            v_sb = [v_p.tile([128, H * (HD + 1)], ESD, tag=f"v{ti}", name=f"v{ti}_{b}") for ti in range(len(TT))]
            for ti, (ts_, P) in enumerate(TT):
                vv = v_sb[ti].rearrange("p (h c) -> p h c", c=HD + 1)
                nc.vector.tensor_copy(vv[0:P, :, HD:HD + 1], ones_col[0:P, :].to_broadcast((P, H, 1)))
                for ci, (cs, cw) in enumerate(ECH):
                    pm = ps_mm.tile([128, 512], F32, tag="ps_mm", name=f"pmb_{b}_{ti}_{ci}")
                    for dti in range(DT):
                        nc.tensor.matmul(pm[0:P, 0:cw],
                                         xS[dti][:, ts_:ts_ + P],
                                         vwT[dti][:, cs:cs + cw],
                                         start=(dti == 0), stop=(dti == DT - 1))
                    for hh in range(6):
                        h = ci * 6 + hh
                        nc.vector.tensor_copy(v_sb[ti][0:P, h * (HD + 1):h * (HD + 1) + HD],
                                              pm[0:P, hh * HD:(hh + 1) * HD])

            state[b] = (xT, xS, qkT, v_sb)

        def attn(b):
            x0 = b * T
            xT, xS, qkT, v_sb = state.pop(b)
            # attention
            oT = [oT_p.tile([128, TP], SD, tag=f"oT{dti}", name=f"oT{dti}_{b}") for dti in range(DT)]
            rdr_den = drp.tile([12, TP], F32, tag="rdr_den", name=f"rdrden_{b}")
            for h in range(H):
                g, par = h // 2, (h % 2) * 64
                qt = qkT[g]
                kt = qkT[6 + g]
                es = [es_p.tile([128, TP], ESD, tag=f"es{ji}_{h % (3 if (deep or attn16) else 2)}", name=f"es{ji}_{b}_{h}") for ji in range(len(TT))]
                for ji, (js, JP) in enumerate(TT):
                    for (cs, cw) in ICH:
                        pss = ps_s.tile([128, 320], F32, tag="ps_s", name=f"pss_{b}_{h}_{ji}_{cs}")
                        nc.tensor.matmul(pss[0:JP, 0:cw],
                                         kt[par:par + 64, js:js + JP],
                                         qt[par:par + 64, cs:cs + cw],
                                         start=True, stop=True)
                        nc.scalar.activation(es[ji][0:JP, cs:cs + cw], pss[0:JP, 0:cw],
                                             AF.Exp, scale=SCALE)
                for (cs, cw), (_, cwv) in zip(ICH, ICHV):
                    po = ps_o.tile([128, 320], F32, tag="ps_o", name=f"po_{b}_{h}_{cs}")
                    for ji, (js, JP) in enumerate(TT):
                        nc.tensor.matmul(po[0:HD + 1, 0:cw],
                                         v_sb[ji][0:JP, h * (HD + 1):(h + 1) * (HD + 1)],
                                         es[ji][0:JP, cs:cs + cw],
                                         start=(ji == 0), stop=(ji == len(TT) - 1))
                    # evac unnormalized o and the denominator row (via partition-0 tile -> DRAM)
                    nc.vector.tensor_copy(oT[g][par:par + 64, cs:cs + cwv], po[0:HD, 0:cwv])
                    dh = nrm_p.tile([1, 320], F32, tag="dh", name=f"dh_{b}_{h}_{cs}")
                    nc.vector.tensor_copy(dh[:, 0:cwv], po[HD:HD + 1, 0:cwv])
                    nc.sync.dma_start(rdr_den[h:h + 1, cs:cs + cwv], dh[:, 0:cwv])

            # batched reciprocal + per-head broadcast + in-place normalize
            den = den_p.tile([12, TP], F32, tag="den", name=f"den_{b}")
            nc.sync.dma_start(den[:, 0:T], rdr_den[:, 0:T])
            rec = den_p.tile([12, TP], F32, tag="rec", name=f"rec_{b}")
            nc.vector.reciprocal(rec[:, 0:T], den[:, 0:T])
            rdr = drp.tile([12, TP], F32, tag="rdr", name=f"rdr_{b}")
            nc.sync.dma_start(rdr[:, 0:T], rec[:, 0:T])
            for h in range(H):
                g, par = h // 2, (h % 2) * 64
                bc = nrm_p.tile([128, TP], F32, tag="bc", name=f"bc_{b}_{h}")
                nc.sync.dma_start(bc[par:par + 64, 0:T], rdr[h:h + 1, 0:T].to_broadcast((64, T)))
                nc.vector.tensor_tensor(oT[g][par:par + 64, 0:T],
                                        oT[g][par:par + 64, 0:T],
                                        bc[par:par + 64, 0:T], mybir.AluOpType.mult)

            # MM4
            for ti, (ts_, P) in enumerate(TT):
                ys = yout.tile([128, D], F32, tag="y_sb", name=f"ys_{b}_{ti}")
                for (cs, cw) in ECH:
                    pm = ps_mm.tile([128, 512], F32, tag="ps_mm", name=f"pmc_{b}_{ti}_{cs}")
                    for dti in range(DT):
                        nc.tensor.matmul(pm[0:P, 0:cw],
                                         oT[dti][:, ts_:ts_ + P],
                                         pwT[dti][:, cs:cs + cw],
                                         start=(dti == 0), stop=False)
                    nc.tensor.matmul(pm[0:P, 0:cw],
                                     ones_row[:, ts_:ts_ + P],
                                     pb_sb[:, cs:cs + cw],
                                     start=False, stop=True)
                    nc.vector.tensor_copy(ys[0:P, cs:cs + cw], pm[0:P, 0:cw])
                nc.sync.dma_start(y_d[x0 + ts_: x0 + ts_ + P, :], ys[0:P, :])


        stage1(0)
        for b in range(1, nbatch):
            stage1(b)
            attn(b - 1)
        attn(nbatch - 1)
    return nc


def host_inputs(x_c, qkv_w, qkv_b, proj_w, proj_b):
    qkwT = np.ascontiguousarray(qkv_w[0:EQK].T)
    vwT = np.ascontiguousarray(qkv_w[EQK:2304].T)
    b_v = qkv_b[EQK:2304]
    pb_aug = proj_b + b_v @ proj_w.T
    pwT = np.concatenate([proj_w.T, pb_aug[None, :]], axis=0)
    qkb = np.ascontiguousarray(qkv_b[0:EQK].reshape(12, 128).T)
    return {
        "x": np.ascontiguousarray(x_c.reshape(NTOK, D)).astype(np.float32),
        "qkwT": qkwT.astype(np.float32),
        "vwT": vwT.astype(np.float32),
        "pwT": pwT.astype(np.float32),
        "qkb": qkb.astype(np.float32),
    }


import sys as _sys
import numpy as _np

def _split_waits(nc, max_waits=1):
    import concourse.mybir as mybir
    nid = [0]
    for f in nc.m.functions:
        for bb in f.blocks:
            newlist = []; changed = False
            for ins in bb.instructions:
                si = getattr(ins, 'sync_info', None)
                if si is not None and si.on_wait is not None and len(si.on_wait) > max_waits:
                    waits = list(si.on_wait)
                    extra, keep = waits[:-max_waits], waits[-max_waits:]
                    for i in range(0, len(extra), max_waits):
                        nop = mybir.InstNoOp(name=f"I-ws-{nid[0]}", ins=[], outs=[],
                            engine=ins.engine,
                            sync_info=mybir.SyncInfo(on_wait=extra[i:i+max_waits], on_update=[]))
                        nid[0] += 1; newlist.append(nop); changed = True
                    si.on_wait = keep; ins.sync_info = si
                newlist.append(ins)
            if changed:
                bb.instructions = newlist


_NC_CACHE = {}

def _get_nc():
    if "nc" not in _NC_CACHE:
        nc = build(nbatch=B, sd="bf16", md="bf16", attn16=True)
        _split_waits(nc)
        _NC_CACHE["nc"] = nc
    return _NC_CACHE["nc"]


def kernel(x, qkv_w, qkv_b, proj_w, proj_b):
    """Full inputs in ([32,577,768] etc.), full output out.

    Data-parallel over batch: 32 batches -> 8 NeuronCores x 4 each. Weights
    replicated (host-side transpose is layout prep only); all compute on
    device (Bass/Tile, fp32r matmuls with fp32 accumulation, software-
    pipelined batch stages).
    """
    x = _np.asarray(x, dtype=_np.float32)
    qkv_w = _np.asarray(qkv_w, dtype=_np.float32)
    qkv_b = _np.asarray(qkv_b, dtype=_np.float32)
    proj_w = _np.asarray(proj_w, dtype=_np.float32)
    proj_b = _np.asarray(proj_b, dtype=_np.float32)
    from concourse.bass_utils import run_bass_kernel_spmd
    nc = _get_nc()
    in_maps = [host_inputs(x[c * B:(c + 1) * B], qkv_w, qkv_b, proj_w, proj_b)
               for c in range(8)]
    res = run_bass_kernel_spmd(nc, in_maps, list(range(8)))
    y = _np.concatenate([res.results[c]["y"].reshape(B, T, D) for c in range(8)], axis=0)
    return y.astype(_np.float32)



# revision 3
# speedup vs baseline: 1.8398x; 1.1565x over previous
"""v6: bf16 operands everywhere (FWL weight loads, 1 cyc/row matmuls);
MM2 head-pair row-packing via tile_position; 578-wide merged exp reads
spanning 2 PSUM banks; bias-free MM4 (DVE broadcast add); exp(-ln(den))
normalization; emission-interleaved stage1(b+1) / attn(b) schedule."""
import numpy as np
import concourse.bass as bass
import concourse.mybir as mybir
import concourse.tile as tile
from concourse.masks import make_identity

dt = mybir.dt
F32 = dt.float32
BF16 = dt.bfloat16
AF = mybir.ActivationFunctionType

B = 4
T = 577
D = 768
H = 12
HD = 64
EQK = 1536
SCALE = HD ** -0.5
NTOK = B * T

TT = [(i * 128, min(128, T - i * 128)) for i in range((T + 127) // 128)]
TP = 578
CH2 = [(0, 512), (512, 66)]     # MM2/MM3 q chunks (bank-aligned for merged ACT)
CH1 = [(0, 512), (512, 66)]     # MM1a token chunks
ECH = [(0, 384), (384, 384)]    # MM1b / MM4 feature chunks
DT = 6


def build(nbatch=B):
    nc = bass.Bass()
    x_d = nc.dram_tensor("x", [NTOK, D], F32, kind="ExternalInput")
    qkwT_d = nc.dram_tensor("qkwT", [D, EQK], F32, kind="ExternalInput")
    vwT_d = nc.dram_tensor("vwT", [D, D], F32, kind="ExternalInput")
    pwT_d = nc.dram_tensor("pwT", [D + 1, D], F32, kind="ExternalInput")
    qkb_d = nc.dram_tensor("qkb", [128, 12], F32, kind="ExternalInput")
    y_d = nc.dram_tensor("y", [NTOK, D], F32, kind="ExternalOutput")

    from contextlib import ExitStack
    with tile.TileContext(nc) as tc, ExitStack() as ctx:
        wpool = ctx.enter_context(tc.tile_pool(name="wpool", bufs=1))
        stg = ctx.enter_context(tc.tile_pool(name="stg", bufs=2))

        ident = wpool.tile([128, 128], BF16, tag="ident")
        make_identity(nc, ident[:])

        zcol = wpool.tile([128, 1], BF16, tag="zcol")
        nc.gpsimd.memset(zcol[:], 0.0)
        ones_col_f = wpool.tile([128, 1], F32, tag="ones_col_f")
        nc.gpsimd.memset(ones_col_f[:], 1.0)
        ones_col = wpool.tile([128, 1], BF16, tag="ones_col")
        nc.vector.tensor_copy(ones_col[:], ones_col_f[:])

        qkb_sb = wpool.tile([128, 12], F32, tag="qkb")
        nc.sync.dma_start(qkb_sb[:], qkb_d[:])
        # proj bias (with v-bias folded in) broadcast to all partitions
        pb_bc = wpool.tile([128, D], F32, tag="pb_bc")
        nc.sync.dma_start(pb_bc[:], pwT_d[D:D + 1, :].to_broadcast((128, D)))

        qkwT = []   # stationary lhsT of MM1a, bf16
        for dti in range(DT):
            w = wpool.tile([128, EQK], BF16, tag=f"qkwT{dti}", name=f"qkwT{dti}")
            for half in range(2):
                s = stg.tile([128, D], F32, tag="wstage", name=f"st{dti}_{half}")
                nc.sync.dma_start(s[:], qkwT_d[dti * 128:(dti + 1) * 128, half * D:(half + 1) * D])
                nc.vector.tensor_copy(w[:, half * D:(half + 1) * D], s[:])
            qkwT.append(w)
        vwT = []    # moving rhs of MM1b, bf16
        for dti in range(DT):
            w = wpool.tile([128, D], BF16, tag=f"vwT{dti}", name=f"vwT{dti}")
            s = stg.tile([128, D], F32, tag="wstage", name=f"sv{dti}")
            nc.sync.dma_start(s[:], vwT_d[dti * 128:(dti + 1) * 128, :])
            nc.vector.tensor_copy(w[:], s[:])
            vwT.append(w)
        pwT = []    # moving rhs of MM4, bf16
        for dti in range(DT):
            w = wpool.tile([128, D], BF16, tag=f"pwT{dti}", name=f"pwT{dti}")
            s = stg.tile([128, D], F32, tag="wstage", name=f"sp{dti}")
            nc.sync.dma_start(s[:], pwT_d[dti * 128:(dti + 1) * 128, :])
            nc.vector.tensor_copy(w[:], s[:])
            pwT.append(w)

        xin = ctx.enter_context(tc.tile_pool(name="xin", bufs=2))
        xT_p = ctx.enter_context(tc.tile_pool(name="xT", bufs=1))
        qkT_p = ctx.enter_context(tc.tile_pool(name="qkT", bufs=2))
        v_p = ctx.enter_context(tc.tile_pool(name="v", bufs=2))
        es_p = ctx.enter_context(tc.tile_pool(name="es", bufs=2))
        oT_p = ctx.enter_context(tc.tile_pool(name="oT", bufs=2))
        nrm_p = ctx.enter_context(tc.tile_pool(name="nrm", bufs=2))
        dh_p = ctx.enter_context(tc.tile_pool(name="dh", bufs=4))
        bc_p = ctx.enter_context(tc.tile_pool(name="bc", bufs=3))
        yout = ctx.enter_context(tc.tile_pool(name="yout", bufs=2))
        drp = ctx.enter_context(tc.tile_pool(name="dr", bufs=2, space="DRAM"))

        ps_s = ctx.enter_context(tc.tile_pool(name="ps_s", bufs=1, space="PSUM"))
        ps_o = ctx.enter_context(tc.tile_pool(name="ps_o", bufs=2, space="PSUM"))
        ps_mm = ctx.enter_context(tc.tile_pool(name="ps_mm", bufs=2, space="PSUM"))

        state = {}

        # ---------- stage1 work units ----------
        def u_ld(b, ti):
            x0 = b * T
            ts_, P = TT[ti]
            xt = xin.tile([128, D], F32, tag="x_in", name=f"x_{b}_{ti}")
            nc.sync.dma_start(xt[0:P, :], x_d[x0 + ts_: x0 + ts_ + P, :])
            xb = xin.tile([128, D], BF16, tag="x_bf", name=f"xb_{b}_{ti}")
            nc.vector.tensor_copy(xb[0:P, :], xt[0:P, :])
            xT = state[b]["xT"]
            for dti in range(DT):
                pt = ps_mm.tile([128, 1024], BF16, tag="mm", name=f"pt_{b}_{ti}_{dti}")
                nc.tensor.transpose(pt[:, 0:P], xb[0:P, dti * 128:(dti + 1) * 128], ident[0:P, 0:P])
                nc.vector.tensor_copy(xT[dti][:, ts_:ts_ + P], pt[:, 0:P])
            if ti == len(TT) - 1:
                for dti in range(DT):
                    nc.vector.tensor_copy(xT[dti][:, T:TP], zcol[:])

        def u_mm1a(b, et):
            xT = state[b]["xT"]
            qkT = state[b]["qkT"]
            for (cs, cw) in CH1:
                pm = ps_mm.tile([128, 512], F32, tag="mm", name=f"pma_{b}_{et}_{cs}")
                for dti in range(DT):
                    nc.tensor.matmul(pm[:, 0:cw],
                                     qkwT[dti][:, et * 128:(et + 1) * 128],
                                     xT[dti][:, cs:cs + cw],
                                     start=(dti == 0), stop=(dti == DT - 1))
                nc.vector.tensor_scalar_add(qkT[et][:, cs:cs + cw], pm[:, 0:cw],
                                            qkb_sb[:, et:et + 1])

        def u_mm1b(b, ti):
            xT = state[b]["xT"]
            v_sb = state[b]["v"]
            ts_, P = TT[ti]
            vv = v_sb[ti].rearrange("p (h c) -> p h c", c=HD + 1)
            nc.vector.tensor_copy(vv[0:P, :, HD:HD + 1], ones_col[0:P, :].to_broadcast((P, H, 1)))
            for ci, (cs, cw) in enumerate(ECH):
                pm = ps_mm.tile([128, 512], F32, tag="mm", name=f"pmb_{b}_{ti}_{ci}")
                for dti in range(DT):
                    nc.tensor.matmul(pm[0:P, 0:cw],
                                     xT[dti][:, ts_:ts_ + P],
                                     vwT[dti][:, cs:cs + cw],
                                     start=(dti == 0), stop=(dti == DT - 1))
                pmv = pm.rearrange("p (h c) -> p h c", c=HD)
                nc.vector.tensor_copy(vv[0:P, ci * 6:(ci + 1) * 6, 0:HD],
                                      pmv[0:P, 0:6, :])

        def stage1_units(b):
            state[b] = {
                "xT": [xT_p.tile([128, TP], BF16, tag=f"xT{dti}", name=f"xT{dti}_{b}") for dti in range(DT)],
                "qkT": [qkT_p.tile([128, TP], BF16, tag=f"qkT{et}", name=f"qkT{et}_{b}") for et in range(12)],
                "v": [v_p.tile([128, H * (HD + 1)], BF16, tag=f"v{ti}", name=f"v{ti}_{b}") for ti in range(len(TT))],
            }
            units = [(lambda b=b, ti=ti: u_ld(b, ti)) for ti in range(len(TT))]
            units += [(lambda b=b, et=et: u_mm1a(b, et)) for et in range(12)]
            units += [(lambda b=b, ti=ti: u_mm1b(b, ti)) for ti in range(len(TT))]
            return units

        # ---------- attention ----------
        def mm2_ji(b, g, ji):
            """score matmuls + merged exp for head pair (2g, 2g+1), k-tile ji"""
            st = state[b]
            qt, kt = st["qkT"][g], st["qkT"][6 + g]
            js, JP = TT[ji]
            pss = []
            for hp in range(2):
                par = hp * 64
                p = ps_s.tile([128, TP], F32, tag=f"s{hp}", name=f"pss_{b}_{g}_{ji}_{hp}")
                pss.append(p)
            # interleave the two heads' matmuls: disjoint row groups -> concurrent
            for (cs, cw) in CH2:
                for hp in range(2):
                    par = hp * 64
                    nc.tensor.matmul(pss[hp][0:JP, cs:cs + cw],
                                     kt[par:par + 64, js:js + JP],
                                     qt[par:par + 64, cs:cs + cw],
                                     start=True, stop=True)
            for hp in range(2):
                es = es_p.tile([128, TP], BF16, tag=f"es{ji}_{hp}", name=f"es_{b}_{g}_{ji}_{hp}")
                nc.scalar.activation(es[0:JP, 0:TP], pss[hp][0:JP, 0:TP], AF.Exp, scale=SCALE)
                st.setdefault("es", {})[(g, ji, hp)] = es

        def u_mm3(b, g, hp, ci):
            """attn @ v for head 2g+hp, output chunk ci"""
            st = state[b]
            h = 2 * g + hp
            par = (h % 2) * 64
            cs, cw = CH2[ci]
            cwv = cw if ci == 0 else cw - 1   # drop padded q column 577
            po = ps_o.tile([128, 512], F32, tag="o", name=f"po_{b}_{h}_{ci}")
            for ji in range(len(TT)):
                js, JP = TT[ji]
                es = st["es"][(g, ji, hp)]
                nc.tensor.matmul(po[0:HD + 1, 0:cw],
                                 st["v"][ji][0:JP, h * (HD + 1):(h + 1) * (HD + 1)],
                                 es[0:JP, cs:cs + cw],
                                 start=(ji == 0), stop=(ji == len(TT) - 1))
            nc.vector.tensor_copy(st["oT"][g][par:par + 64, cs:cs + cwv], po[0:HD, 0:cwv])
            dh = st["dh"][h]
            nc.vector.tensor_copy(dh[:, cs:cs + cwv], po[HD:HD + 1, 0:cwv])
            if ci == len(CH2) - 1:
                nc.sync.dma_start(st["rdr_den"][h:h + 1, 0:T], dh[:, 0:T])

        def attn_norm(b):
            """reciprocal of softmax denominators via exp(-ln(den)); broadcast; normalize"""
            st = state[b]
            den = nrm_p.tile([12, TP], F32, tag="den", name=f"den_{b}")
            nc.sync.dma_start(den[:, 0:T], st["rdr_den"][:, 0:T])
            lnd = nrm_p.tile([12, TP], F32, tag="lnd", name=f"lnd_{b}")
            nc.scalar.activation(lnd[:, 0:T], den[:, 0:T], AF.Ln)
            rec = nrm_p.tile([12, TP], BF16, tag="rec", name=f"rec_{b}")
            nc.scalar.activation(rec[:, 0:T], lnd[:, 0:T], AF.Exp, scale=-1.0)
            rdr2 = drp.tile([12, TP], BF16, tag="rdr2", name=f"rdr2_{b}")
            nc.sync.dma_start(rdr2[:, 0:T], rec[:, 0:T])
            for g in range(6):
                bc = bc_p.tile([128, TP], BF16, tag="bc", name=f"bc_{b}_{g}")
                for hp in range(2):
                    par = hp * 64
                    nc.sync.dma_start(bc[par:par + 64, 0:T],
                                      rdr2[2 * g + hp:2 * g + hp + 1, 0:T].to_broadcast((64, T)))
                nc.vector.tensor_tensor(st["oT"][g][:, 0:T], st["oT"][g][:, 0:T],
                                        bc[:, 0:T], mybir.AluOpType.mult)

        def u_mm4(b, ti):
            st = state[b]
            x0 = b * T
            ts_, P = TT[ti]
            ys = yout.tile([128, D], F32, tag="y_sb", name=f"ys_{b}_{ti}")
            for (cs, cw) in ECH:
                pm = ps_mm.tile([128, 512], F32, tag="mm", name=f"pmc_{b}_{ti}_{cs}")
                for dti in range(DT):
                    nc.tensor.matmul(pm[0:P, 0:cw],
                                     st["oT"][dti][:, ts_:ts_ + P],
                                     pwT[dti][:, cs:cs + cw],
                                     start=(dti == 0), stop=(dti == DT - 1))
                nc.vector.tensor_tensor(ys[0:P, cs:cs + cw], pm[0:P, 0:cw],
                                        pb_bc[0:P, cs:cs + cw], mybir.AluOpType.add)
            nc.sync.dma_start(y_d[x0 + ts_: x0 + ts_ + P, :], ys[0:P, :])

        def attn_emit(b, unit_queue):
            """emit attn(b) pairs, interleaving MM3 of the previous pair and
            pulled units (stage1 of b+1 / MM4 of b-1) into PE idle slots"""
            st = state[b]
            st["oT"] = [oT_p.tile([128, TP], BF16, tag=f"oT{dti}", name=f"oT{dti}_{b}") for dti in range(DT)]
            st["dh"] = [dh_p.tile([1, TP], F32, tag="dh", name=f"dh_{b}_{h}") for h in range(H)]
            st["rdr_den"] = drp.tile([12, TP], F32, tag="rdr_den", name=f"rdrden_{b}")

            def pull(k):
                for _ in range(k):
                    if unit_queue:
                        unit_queue.pop(0)()

            for g in range(6):
                mm3_units = []
                if g > 0:
                    mm3_units = [(g - 1, hp, ci) for hp in range(2) for ci in range(len(CH2))]
                for ji in range(len(TT)):
                    mm2_ji(b, g, ji)
                    if mm3_units:
                        pg, hp, ci = mm3_units.pop(0)
                        u_mm3(b, pg, hp, ci)
                        pull(1)
                    else:
                        pull(1 if g == 0 else 2)
                pull(1)
            # last pair's MM3 + normalization
            for hp in range(2):
                for ci in range(len(CH2)):
                    u_mm3(b, 5, hp, ci)
                    pull(1)
            attn_norm(b)

        # ---------- schedule ----------
        for u in stage1_units(0):
            u()
        for b in range(nbatch):
            queue = []
            if b + 1 < nbatch:
                queue += stage1_units(b + 1)
            if b > 0:
                queue += [(lambda b=b, ti=ti: u_mm4(b - 1, ti)) for ti in range(len(TT))]
            attn_emit(b, queue)
            for u in queue:   # drain anything not pulled
                u()
            if b == nbatch - 1:
                for ti in range(len(TT)):
                    u_mm4(b, ti)
            state.pop(b - 1, None)
    return nc


def host_inputs(x_c, qkv_w, qkv_b, proj_w, proj_b):
    qkwT = np.ascontiguousarray(qkv_w[0:EQK].T)
    vwT = np.ascontiguousarray(qkv_w[EQK:2304].T)
    b_v = qkv_b[EQK:2304]
    pb_aug = proj_b + b_v @ proj_w.T
    pwT = np.concatenate([proj_w.T, pb_aug[None, :]], axis=0)
    qkb = np.ascontiguousarray(qkv_b[0:EQK].reshape(12, 128).T)
    return {
        "x": np.ascontiguousarray(x_c.reshape(NTOK, D)).astype(np.float32),
        "qkwT": qkwT.astype(np.float32),
        "vwT": vwT.astype(np.float32),
        "pwT": pwT.astype(np.float32),
        "qkb": qkb.astype(np.float32),
    }


import sys as _sys
import numpy as _np

def _split_waits(nc, max_waits=1):
    import concourse.mybir as mybir
    nid = [0]
    for f in nc.m.functions:
        for bb in f.blocks:
            newlist = []; changed = False
            for ins in bb.instructions:
                si = getattr(ins, 'sync_info', None)
                if si is not None and si.on_wait is not None and len(si.on_wait) > max_waits:
                    waits = list(si.on_wait)
                    extra, keep = waits[:-max_waits], waits[-max_waits:]
                    for i in range(0, len(extra), max_waits):
                        nop = mybir.InstNoOp(name=f"I-ws-{nid[0]}", ins=[], outs=[],
                            engine=ins.engine,
                            sync_info=mybir.SyncInfo(on_wait=extra[i:i+max_waits], on_update=[]))
                        nid[0] += 1; newlist.append(nop); changed = True
                    si.on_wait = keep; ins.sync_info = si
                newlist.append(ins)
            if changed:
                bb.instructions = newlist


_NC_CACHE = {}

def _get_nc():
    if "nc" not in _NC_CACHE:
        nc = build(nbatch=B)
        _split_waits(nc)
        _NC_CACHE["nc"] = nc
    return _NC_CACHE["nc"]


def kernel(x, qkv_w, qkv_b, proj_w, proj_b):
    """Full inputs in ([32,577,768] etc.), full output out.

    Data-parallel over batch: 32 batches -> 8 NeuronCores x 4 each. Weights
    replicated (host-side transpose is layout prep only); all compute on
    device (Bass/Tile, bf16 matmuls with fp32 accumulation, software-
    pipelined batch stages).
    """
    x = _np.asarray(x, dtype=_np.float32)
    qkv_w = _np.asarray(qkv_w, dtype=_np.float32)
    qkv_b = _np.asarray(qkv_b, dtype=_np.float32)
    proj_w = _np.asarray(proj_w, dtype=_np.float32)
    proj_b = _np.asarray(proj_b, dtype=_np.float32)
    from concourse.bass_utils import run_bass_kernel_spmd
    nc = _get_nc()
    in_maps = [host_inputs(x[c * B:(c + 1) * B], qkv_w, qkv_b, proj_w, proj_b)
               for c in range(8)]
    res = run_bass_kernel_spmd(nc, in_maps, list(range(8)))
    y = _np.concatenate([res.results[c]["y"].reshape(B, T, D) for c in range(8)], axis=0)
    return y.astype(_np.float32)
